# revision 17
# baseline (speedup 1.0000x reference)
"""Trainium2 Bass kernel for nn_MultiHeadCrossAttn.

Contract: kernel(**inputs) takes the FULL unsharded inputs (as produced by
reference.setup_inputs()) and returns the full outputs matching
reference.reference(**inputs):
  (g_vec, d_vec, out_g_tokens, out_d_tokens, scores_g, attn_g, scores_d, attn_d)

Strategy: pure data parallel over batch B=256 -> 8 NeuronCores x 32 batches.
All parameters replicated. One Bass/Tile program per core (SPMD).

Key implementation notes:
 - LN affine params of ln_node / ln_desc / tok_ln are folded into the
   downstream projection weights host-side; the kernel computes pure
   normalize (x-mu)*rsqrt(var+eps). 1/sqrt(DH) is folded into Wq.
 - Per-column biases (bv_*, proj_*_b) are folded into a single column bias
   accumulated into the output PSUM via a K=1 ones-matmul (attn rows sum
   to 1, so ctx = attn@(V+bv) = attn@V + bv).
 - rsqrt is computed as exp(-0.5*ln(var+eps)) so the ACT engine only ever
   uses the natural_log_exp_and_others table set (no table-swap cost).
 - Key-pad mask: host sends rows of 0/-3.4e38; two f32r K=1 ones-matmul
   accumulations overflow masked score columns to exactly -inf in PSUM
   (PE multiplication of +/-inf gives NaN, so -inf must be created by
   accumulation overflow).  exp(-inf) = 0 on the ACT spline (verified).
 - f32r (4x faster than f32 for moving dim >= 256) is used for matmuls
   whose output free dim is 256/512 and for attention-weight transposes.
"""

import numpy as np

import concourse.bacc as bacc
import concourse.bass as bass
import concourse.mybir as mybir
import concourse.tile as tile
from concourse.bass_utils import run_bass_kernel_spmd

F32 = mybir.dt.float32
F32R = mybir.dt.float32r
AF = mybir.ActivationFunctionType
ALU = mybir.AluOpType
AX = mybir.AxisListType

B, N, D = 256, 128, 200
DG = DT = DK = 256
H, DH = 4, 64
NCORES = 8
BB = B // NCORES            # 32 batches per core
D1 = D - 128                # 72  (second desc-token partition chunk)
EPS = 1e-5
BIGNEG = float(np.float32(-3.4e38))

LAST_RESULTS = None         # BassKernelResults of the most recent run (for test.py)
_PROGRAM_CACHE = {}


def _f(ap):
    """f32 view of an f32r AP for DVE/ACT reads (same bits)."""
    return ap.bitcast(F32) if ap.dtype == F32R else ap


def _rep4(row_ap):
    """Repeat a (1, 128) AP 4x along free dim -> (1, 4, 128) stride-0 AP."""
    return bass.AP(tensor=row_ap.tensor, offset=row_ap.offset,
                   ap=[row_ap.ap[0], [0, 4], row_ap.ap[-1]])


def _bcast_free(ap, n):
    """Broadcast last (scalar) free dim of an AP n-ways (stride 0)."""
    return bass.AP(tensor=ap.tensor, offset=ap.offset,
                   ap=list(ap.ap[:-1]) + [[0, n]])


def build_program(tok_aff, og_aff, od_aff, bb=BB, upto=99):
    """Build the per-core Bass program. Flags select the (rare) general
    paths where LN affine params cannot be folded into weights."""
    nc = bacc.Bacc("TRN2", target_bir_lowering=False)

    # ---------------- DRAM I/O ----------------
    BBx = bb
    d_nt = nc.dram_tensor("nt", [BBx, N, DG], F32, kind="ExternalInput")
    d_desc = nc.dram_tensor("desc", [1, BBx, D], F32R, kind="ExternalInput")
    d_madd = nc.dram_tensor("madd", [1, BBx, N], F32, kind="ExternalInput")
    d_valid = nc.dram_tensor("valid", [N, BBx], F32R, kind="ExternalInput")
    d_rden = nc.dram_tensor("rden", [1, BBx], F32, kind="ExternalInput")

    wnames = ["wq_g", "wk_g", "wv_g", "wq_d", "wk_d", "wv_d", "pnw", "pdw"]
    d_w = {n: nc.dram_tensor(n, [2 * 128, 256], F32R, kind="ExternalInput")
           for n in wnames}
    d_bq = {n: nc.dram_tensor("b" + n, [2 * 128, 1], F32, kind="ExternalInput")
            for n in ["q_g", "k_g", "q_d", "k_d"]}
    d_cbg = nc.dram_tensor("cbg", [1, 256], F32R, kind="ExternalInput")
    d_cbd = nc.dram_tensor("cbd", [1, 256], F32R, kind="ExternalInput")
    d_vw1 = nc.dram_tensor("vw1", [1, 128], F32R, kind="ExternalInput")
    d_vw2 = nc.dram_tensor("vw2", [128, 256], F32R, kind="ExternalInput")
    d_b1 = nc.dram_tensor("b1", [128, 1], F32, kind="ExternalInput")
    d_id2a = nc.dram_tensor("id2a", [128, 256], F32, kind="ExternalInput")
    d_id2b = nc.dram_tensor("id2b", [D1, 256], F32, kind="ExternalInput")
    d_ones = nc.dram_tensor("ones", [128, 512], F32R, kind="ExternalInput")
    d_eye = nc.dram_tensor("eye", [128, 128], F32, kind="ExternalInput")
    d_eyer = nc.dram_tensor("eyer", [128, 128], F32R, kind="ExternalInput")
    d_aff = {}
    for flag, names in [(tok_aff, ["tokg", "tokb"]),
                        (og_aff, ["ogg", "ogb"]),
                        (od_aff, ["odg", "odb"])]:
        if flag:
            for nme in names:
                d_aff[nme] = nc.dram_tensor(nme, [1, 256], F32,
                                            kind="ExternalInput")

    d_gvec = nc.dram_tensor("g_vec", [BBx, 256], F32, kind="ExternalOutput")
    d_dvec = nc.dram_tensor("d_vec", [BBx, 256], F32, kind="ExternalOutput")
    d_outg = nc.dram_tensor("out_g", [BBx, N, 256], F32R, kind="ExternalOutput")
    d_outd = nc.dram_tensor("out_d", [BBx, D, 256], F32R, kind="ExternalOutput")
    d_scg = nc.dram_tensor("scores_g", [BBx, H, N, D], F32, kind="ExternalOutput")
    d_atg = nc.dram_tensor("attn_g", [BBx, H, N, D], F32R, kind="ExternalOutput")
    d_scd = nc.dram_tensor("scores_d", [BBx, H, D, N], F32, kind="ExternalOutput")
    d_atd = nc.dram_tensor("attn_d", [BBx, H, D, N], F32R, kind="ExternalOutput")

    with tile.TileContext(nc) as tc:
        with (
            tc.tile_pool(name="singles", bufs=1) as sg,
            tc.tile_pool(name="work", bufs=2) as wk,
            tc.tile_pool(name="ppout", bufs=2, space="PSUM") as ppout,
            tc.tile_pool(name="ppmm", bufs=2, space="PSUM") as ppmm,
        ):
            # ---------------- static loads ----------------
            w = {}
            for nme in wnames:
                t = sg.tile([128, 2, 256], F32R, tag="w_" + nme)
                nc.sync.dma_start(out=t, in_=d_w[nme].rearrange(
                    "(c p) n -> p c n", p=128))
                w[nme] = t
            bq = {}
            for nme in d_bq:
                t = sg.tile([128, 2, 1], F32, tag="b_" + nme)
                nc.sync.dma_start(out=t, in_=d_bq[nme].rearrange(
                    "(c p) o -> p c o", p=128))
                bq[nme] = t
            cbg = sg.tile([1, 256], F32R, tag="cbg")
            nc.sync.dma_start(out=cbg, in_=d_cbg[:, :])
            cbd = sg.tile([1, 256], F32R, tag="cbd")
            nc.sync.dma_start(out=cbd, in_=d_cbd[:, :])
            vw1 = sg.tile([1, 128], F32R, tag="vw1")
            nc.sync.dma_start(out=vw1, in_=d_vw1[:, :])
            vw2 = sg.tile([128, 256], F32R, tag="vw2")
            nc.sync.dma_start(out=vw2, in_=d_vw2[:, :])
            b1 = sg.tile([128, 1], F32, tag="b1")
            nc.sync.dma_start(out=b1, in_=d_b1[:, :])
            id2a = sg.tile([128, 256], F32, tag="id2a")
            nc.sync.dma_start(out=id2a, in_=d_id2a[:, :])
            id2b = sg.tile([128, 256], F32, tag="id2b")
            nc.sync.dma_start(out=id2b[:D1, :], in_=d_id2b[:, :])
            aff = {}
            for nme, dt_ in d_aff.items():
                t = sg.tile([128, 256], F32, tag="aff_" + nme)
                nc.sync.dma_start(out=t, in_=bass.AP(
                    tensor=dt_[:, :].tensor, offset=dt_[:, :].offset,
                    ap=[[0, 128], [1, 256]]))
                aff[nme] = t

            ident = sg.tile([128, 128], F32, tag="ident")
            nc.sync.dma_start(out=ident, in_=d_eye[:, :])
            onesr = sg.tile([1, 512], F32R, tag="onesr")
            nc.sync.dma_start(out=onesr, in_=d_ones[0:1, :])
            onesc = sg.tile([128, 1], F32R, tag="onesc")
            nc.sync.dma_start(out=onesc, in_=d_ones[:, 0:1])
            epst = sg.tile([128, 1], F32, tag="epst")
            nc.vector.memset(epst, EPS)

            ntall = sg.tile([128, BBx, 256], F32, tag="ntall")
            nc.sync.dma_start(out=ntall, in_=d_nt.rearrange("b n d -> n b d"))
            validt = sg.tile([128, BBx], F32R, tag="validt")
            nc.sync.dma_start(out=validt, in_=d_valid[:, :])
            rden = sg.tile([1, BBx], F32, tag="rden")
            nc.sync.dma_start(out=rden, in_=d_rden[:, :])

            identr = sg.tile([128, 128], F32R, tag="identr")
            nc.sync.dma_start(out=identr, in_=d_eyer[:, :])

            # LN helper: stats of x -> (mu, rstd) scalar APs.
            # mv tile layout (P, k, 2); rstd written into rs (P, k).
            def ln_stats(x_slices, tag):
                """x_slices: list of (ap, P) per chunk. Returns list of
                (mu_ap, rstd_ap) per chunk."""
                k = len(x_slices)
                st = wk.tile([128, k, 6], F32, tag=tag + "_st")
                mv = wk.tile([128, k, 2], F32, tag=tag + "_mv")
                for c, (xap, p) in enumerate(x_slices):
                    if p < 128:
                        # fill the unused partition range so the packed
                        # Ln/Exp below never reads uninitialized memory
                        # (start partition must be 32-aligned; bn_aggr
                        # overwrites [:p] afterwards)
                        nc.vector.memset(mv[(p // 32) * 32:, c, :], 1.0)
                    nc.vector.bn_stats(out=st[:p, c, :], in_=_f(xap))
                    nc.vector.bn_aggr(out=mv[:p, c, :], in_=st[:p, c, :])
                lnv = wk.tile([128, k], F32, tag=tag + "_lnv")
                rs = wk.tile([128, k], F32, tag=tag + "_rs")
                nc.scalar.activation(out=lnv, in_=mv[:, :, 1], func=AF.Ln,
                                     bias=epst, scale=1.0)
                nc.scalar.activation(out=rs, in_=lnv, func=AF.Exp, scale=-0.5)
                return [(mv[:p, c, 0:1], rs[:p, c:c + 1])
                        for c, (xap, p) in enumerate(x_slices)]

            def ln_norm(out_ap, x_ap, mu, rstd):
                nc.vector.tensor_scalar(out=out_ap, in0=_f(x_ap), scalar1=mu,
                                        scalar2=rstd, op0=ALU.subtract,
                                        op1=ALU.mult)

            # ---------------- per-batch emission ----------------
            for b in range(BBx):
                xt = ntall[:, b, :]                       # (128, 256) raw nodes

                # --- node LN (pure normalize; affine folded into weights) ---
                stats_n = ln_stats([(xt, 128)], f"lnN{b % 2}")
                xn = wk.tile([128, 256], F32R, tag="xn")
                ln_norm(xn, xt, *stats_n[0])

                # --- transposes of raw nodes and node_in ---
                ntT_ps = ppmm.tile([128, 2, 128], F32, tag="mm")
                for c in range(2):
                    nc.tensor.transpose(ntT_ps[:, c, :],
                                        xt[:, c * 128:(c + 1) * 128], ident)
                ntT = wk.tile([128, 2, 128], F32R, tag="ntT")
                nc.scalar.copy(ntT, ntT_ps)

                ninT_ps = ppmm.tile([128, 2, 128], F32R, tag="mm")
                for c in range(2):
                    nc.tensor.transpose(ninT_ps[:, c, :],
                                        xn[:, c * 128:(c + 1) * 128], identr)
                ninT = wk.tile([128, 2, 128], F32R, tag="ninT")
                nc.vector.tensor_copy(ninT, _f(ninT_ps))

                if upto < 2:
                    continue
                # --- descriptor value-MLP ---
                drow = wk.tile([1, D], F32R, tag="drow")
                nc.sync.dma_start(out=drow, in_=d_desc[0:1, b, :])
                h_ps = ppmm.tile([128, D], F32, tag="mm")
                nc.tensor.matmul(h_ps, vw1, drow, start=True, stop=True)
                hT = wk.tile([128, D], F32R, tag="hT")
                nc.scalar.activation(out=hT, in_=h_ps, func=AF.Relu, bias=b1,
                                     scale=1.0)

                ve_ps = ppmm.tile([128, 2, 256], F32, tag="mm")
                nc.tensor.matmul(ve_ps[:, 0, :], hT[:, 0:128], vw2,
                                 start=True, stop=True)
                nc.tensor.matmul(ve_ps[:D1, 1, :], hT[:, 128:D], vw2,
                                 start=True, stop=True)
                xtok = wk.tile([128, 2, 256], F32, tag="xtok")
                nc.vector.tensor_add(xtok[:, 0, :], ve_ps[:, 0, :], id2a)
                nc.vector.tensor_add(xtok[:D1, 1, :], ve_ps[:D1, 1, :],
                                     id2b[:D1, :])

                # --- tok LN -> desc_tks (normalized) ---
                stats_t = ln_stats([(xtok[:, 0, :], 128),
                                    (xtok[:D1, 1, :], D1)], f"lnT{b % 2}")
                dtk = wk.tile([128, 2, 256], F32R, tag="dtk")
                ln_norm(dtk[:, 0, :], xtok[:, 0, :], *stats_t[0])
                ln_norm(dtk[:D1, 1, :], xtok[:D1, 1, :], *stats_t[1])

                if tok_aff:
                    dtka = wk.tile([128, 2, 256], F32, tag="dtka")
                    for c, p in [(0, 128), (1, D1)]:
                        nc.vector.tensor_mul(dtka[:p, c, :], _f(dtk[:p, c, :]),
                                             aff["tokg"][:p, :])
                        nc.vector.tensor_add(dtka[:p, c, :], dtka[:p, c, :],
                                             aff["tokb"][:p, :])
                    din_src = dtka
                else:
                    din_src = dtk

                # --- desc_tks^T (for proj_desc; tok affine folded into pdw) ---
                dtkT_ps = ppmm.tile([128, 2, 200], F32R, tag="mm")
                for c in range(2):
                    nc.tensor.transpose(dtkT_ps[:, c, 0:128],
                                        dtk[:, 0, c * 128:(c + 1) * 128], identr)
                    nc.tensor.transpose(dtkT_ps[:, c, 128:D],
                                        dtk[:D1, 1, c * 128:(c + 1) * 128],
                                        identr[:D1, :D1])
                dtkT = wk.tile([128, 2, 200], F32R, tag="dtkT")
                nc.scalar.copy(dtkT, _f(dtkT_ps))

                # --- desc_in = LN(desc_tks) (normalized; affine folded) ---
                stats_d = ln_stats([(din_src[:, 0, :], 128),
                                    (din_src[:D1, 1, :], D1)], f"lnD{b % 2}")
                din = wk.tile([128, 2, 256], F32R, tag="din")
                ln_norm(din[:, 0, :], din_src[:, 0, :], *stats_d[0])
                ln_norm(din[:D1, 1, :], din_src[:D1, 1, :], *stats_d[1])

                dinT_ps = ppmm.tile([128, 2, 200], F32R, tag="mm")
                for c in range(2):
                    nc.tensor.transpose(dinT_ps[:, c, 0:128],
                                        din[:, 0, c * 128:(c + 1) * 128], identr)
                    nc.tensor.transpose(dinT_ps[:, c, 128:D],
                                        din[:D1, 1, c * 128:(c + 1) * 128],
                                        identr[:D1, :D1])
                dinT = wk.tile([128, 2, 200], F32R, tag="dinT")
                nc.vector.tensor_copy(dinT, _f(dinT_ps))

                if upto < 3:
                    continue
                # --- projections (transposed Q/K; natural V) ---
                def projT(wt, bias_t, rhs_free, rhs_tile, tag, act=False):
                    """Q^T/K^T: out (128, 2, rhs_free) sbuf; lhsT = W chunks."""
                    ps = ppmm.tile([128, 2, rhs_free], F32, tag="mm")
                    for mc in range(2):
                        for kc in range(2):
                            nc.tensor.matmul(
                                ps[:, mc, :],
                                w[wt][:, kc, mc * 128:(mc + 1) * 128],
                                rhs_tile[:, kc, :],
                                start=(kc == 0), stop=(kc == 1))
                    out = wk.tile([128, 2, rhs_free], F32R, tag=tag)
                    for mc in range(2):
                        nc.vector.tensor_scalar_add(out[:, mc, :],
                                                    ps[:, mc, :],
                                                    bias_t[:, mc, :])
                    return out

                QgT = projT("wq_g", bq["q_g"], 128, ninT, "QgT")
                KgT = projT("wk_g", bq["k_g"], 200, dinT, "KgT")
                QdT = projT("wq_d", bq["q_d"], 200, dinT, "QdT")
                KdT = projT("wk_d", bq["k_d"], 128, ninT, "KdT")

                # V_g natural (200,256) in chunks; V_d natural (128,256)
                vg_ps = ppmm.tile([128, 2, 256], F32, tag="mm")
                for mc, p, msl in [(0, 128, slice(0, 128)),
                                   (1, D1, slice(128, D))]:
                    for kc in range(2):
                        nc.tensor.matmul(vg_ps[:p, mc, :],
                                         dinT[:, kc, msl],
                                         w["wv_g"][:, kc, :],
                                         start=(kc == 0), stop=(kc == 1))
                Vg = wk.tile([128, 2, 256], F32R, tag="Vg")
                nc.scalar.copy(Vg[:, 0, :], vg_ps[:, 0, :])
                nc.scalar.copy(Vg[:D1, 1, :], vg_ps[:D1, 1, :])

                vd_ps = ppmm.tile([128, 256], F32, tag="mm")
                for kc in range(2):
                    nc.tensor.matmul(vd_ps, ninT[:, kc, :],
                                     w["wv_d"][:, kc, :],
                                     start=(kc == 0), stop=(kc == 1))
                Vd = wk.tile([128, 256], F32R, tag="Vd")
                nc.scalar.copy(Vd, vd_ps)

                # --- output projections into PSUM (ctx accumulates later) ---
                pn_ps = ppout.tile([128, 256], F32, tag="pn")
                for kc in range(2):
                    nc.tensor.matmul(pn_ps, ntT[:, kc, :],
                                     w["pnw"][:, kc, :],
                                     start=(kc == 0), stop=False,
                                     skip_group_check=True)
                nc.tensor.matmul(pn_ps, onesr[0:1, 0:128], cbg,
                                 start=False, stop=False,
                                 skip_group_check=True)

                pd_ps = [ppout.tile([128, 256], F32, tag="pd0", name="pd0"),
                         ppout.tile([128, 256], F32, tag="pd1", name="pd1")]
                for mc, p in [(0, 128), (1, D1)]:
                    for kc in range(2):
                        nc.tensor.matmul(
                            pd_ps[mc][:p, :],
                            dtkT[:, kc, mc * 128:mc * 128 + p],
                            w["pdw"][:, kc, :],
                            start=(kc == 0), stop=False,
                            skip_group_check=True)
                    nc.tensor.matmul(pd_ps[mc][:p, :], onesr[0:1, 0:p],
                                     cbd, start=False, stop=False,
                                     skip_group_check=True)

                if upto < 4:
                    continue
                # ---------------- attention: nodes -> desc (g) ----------------
                scg = wk.tile([128, 4, 200], F32, tag="scg")
                for h in range(4):
                    hp = (h % 2) * 64
                    ps_h = ppmm.tile([128, 200], F32, tag="mm",
                                     name=f"scg_ps{h}")
                    nc.tensor.matmul(ps_h,
                                     QgT[hp:hp + 64, h // 2, :],
                                     KgT[hp:hp + 64, h // 2, :],
                                     start=True, stop=True)
                    if h % 2 == 0:
                        nc.vector.tensor_copy(scg[:, h, :], ps_h)
                    else:
                        nc.scalar.copy(scg[:, h, :], ps_h)
                nc.sync.dma_start(
                    out=d_scg[b].rearrange("h q k -> q h k"), in_=scg)

                if upto < 5:
                    continue
                atg = wk.tile([128, 4, 200], F32R, tag="atg")
                nc.scalar.activation(out=atg, in_=scg, func=AF.Exp)
                smg = wk.tile([128, 4, 1], F32, tag="smg")
                nc.vector.reduce_sum(smg, _f(atg), axis=AX.X)
                rsg = wk.tile([128, 4, 1], F32, tag="rsg")
                nc.vector.reciprocal(rsg, smg)
                nc.vector.tensor_mul(atg, _f(atg), _bcast_free(rsg, 200))
                nc.sync.dma_start(
                    out=d_atg[b].rearrange("h q k -> q h k"), in_=atg)

                if upto < 6:
                    continue
                agT_ps = [ppmm.tile([128, 4, 128], F32R, tag="mm",
                                    name=f"agT_ps{i}") for i in range(2)]
                for h in range(4):
                    nc.tensor.transpose(agT_ps[0][:, h, :],
                                        atg[:, h, 0:128], identr)
                    nc.tensor.transpose(agT_ps[1][:D1, h, :],
                                        atg[:, h, 128:D], identr)
                agT0 = wk.tile([128, 4, 128], F32R, tag="agT0")
                nc.scalar.copy(agT0, _f(agT_ps[0]))
                agT1 = wk.tile([128, 4, 128], F32R, tag="agT1")
                nc.scalar.copy(agT1[:D1], _f(agT_ps[1][:D1]))

                if upto < 7:
                    continue
                for h in range(4):
                    nc.tensor.matmul(pn_ps[:, h * 64:(h + 1) * 64],
                                     agT0[:, h, :], Vg[:, 0, h * 64:(h + 1) * 64],
                                     start=False, stop=False,
                                     skip_group_check=True)
                    nc.tensor.matmul(pn_ps[:, h * 64:(h + 1) * 64],
                                     agT1[:D1, h, :], Vg[:D1, 1, h * 64:(h + 1) * 64],
                                     start=False, stop=True,
                                     skip_group_check=True)

                # --- out_g = LN(proj + ctx) ---
                stats_og = ln_stats([(pn_ps[:, :], 128)], f"lnG{b % 2}")
                og = wk.tile([128, 256], F32R, tag="og")
                ln_norm(og, pn_ps[:, :], *stats_og[0])
                if og_aff:
                    nc.vector.tensor_mul(og, _f(og), aff["ogg"])
                    nc.vector.tensor_add(og, _f(og), aff["ogb"])
                nc.sync.dma_start(out=d_outg[b], in_=og)

                if upto < 8:
                    continue
                # --- g_vec (masked mean over valid nodes) ---
                gv_ps = ppmm.tile([1, 256], F32, tag="mm")
                nc.tensor.matmul(gv_ps, validt[:, b:b + 1], og,
                                 start=True, stop=True)
                gv = wk.tile([1, 256], F32, tag="gv")
                nc.vector.tensor_scalar_mul(gv, gv_ps, rden[0:1, b:b + 1])
                nc.sync.dma_start(out=d_gvec[b:b + 1, :], in_=gv)

                if upto < 9:
                    continue
                # ---------------- attention: desc -> nodes (d) ----------------
                # -inf additive mask row broadcast across partitions via DMA
                madd_bc = wk.tile([128, N], F32, tag="madd_bc")
                mrow_src = d_madd[0:1, b, :]
                nc.sync.dma_start(out=madd_bc, in_=bass.AP(
                    tensor=mrow_src.tensor, offset=mrow_src.offset,
                    ap=[[0, 128]] + [list(x) for x in mrow_src.ap[1:]]))

                scd0 = wk.tile([128, 4, 128], F32, tag="scd0")
                scd1 = wk.tile([128, 4, 128], F32, tag="scd1")
                for h in range(4):
                    hp = (h % 2) * 64
                    ps_h0 = ppmm.tile([128, 128], F32, tag="mm",
                                      name=f"scd_ps0{h}")
                    nc.tensor.matmul(ps_h0,
                                     QdT[hp:hp + 64, h // 2, 0:128],
                                     KdT[hp:hp + 64, h // 2, :],
                                     start=True, stop=True)
                    ps_h1 = ppmm.tile([128, 128], F32, tag="mm",
                                      name=f"scd_ps1{h}")
                    nc.tensor.matmul(ps_h1[:D1],
                                     QdT[hp:hp + 64, h // 2, 128:D],
                                     KdT[hp:hp + 64, h // 2, :],
                                     start=True, stop=True)
                    # scores + (-inf at masked key cols), fused into the copy
                    nc.vector.tensor_add(scd0[:, h, :], ps_h0, madd_bc)
                    nc.vector.tensor_add(scd1[:D1, h, :], ps_h1[:D1],
                                         madd_bc[:D1])
                nc.sync.dma_start(
                    out=d_scd[b, :, 0:128, :].rearrange("h q k -> q h k"),
                    in_=scd0)
                nc.sync.dma_start(
                    out=d_scd[b, :, 128:D, :].rearrange("h q k -> q h k"),
                    in_=scd1[:D1])

                atd0 = wk.tile([128, 4, 128], F32R, tag="atd0")
                nc.scalar.activation(out=atd0, in_=scd0, func=AF.Exp)
                atd1 = wk.tile([128, 4, 128], F32R, tag="atd1")
                nc.scalar.activation(out=atd1[:D1], in_=scd1[:D1], func=AF.Exp)
                smd0 = wk.tile([128, 4, 1], F32, tag="smd0")
                nc.vector.reduce_sum(smd0, _f(atd0), axis=AX.X)
                smd1 = wk.tile([128, 4, 1], F32, tag="smd1")
                nc.vector.reduce_sum(smd1[:D1], _f(atd1[:D1]), axis=AX.X)
                rsd0 = wk.tile([128, 4, 1], F32, tag="rsd0")
                nc.vector.reciprocal(rsd0, smd0)
                rsd1 = wk.tile([128, 4, 1], F32, tag="rsd1")
                nc.vector.reciprocal(rsd1[:D1], smd1[:D1])
                nc.vector.tensor_mul(atd0, _f(atd0), _bcast_free(rsd0, 128))
                nc.vector.tensor_mul(atd1[:D1], _f(atd1[:D1]),
                                     _bcast_free(rsd1[:D1], 128))
                nc.sync.dma_start(
                    out=d_atd[b, :, 0:128, :].rearrange("h q k -> q h k"),
                    in_=atd0)
                nc.sync.dma_start(
                    out=d_atd[b, :, 128:D, :].rearrange("h q k -> q h k"),
                    in_=atd1[:D1])

                adT_ps = [ppmm.tile([128, 2, 200], F32R, tag="mm",
                                    name=f"adT_ps{i}") for i in range(2)]
                for h in range(4):
                    nc.tensor.transpose(adT_ps[h // 2][:, h % 2, 0:128],
                                        atd0[:, h, :], identr)
                    nc.tensor.transpose(adT_ps[h // 2][:, h % 2, 128:D],
                                        atd1[:D1, h, :],
                                        identr[:D1, :D1])
                adT = wk.tile([128, 4, 200], F32R, tag="adT")
                nc.vector.tensor_copy(adT[:, 0:2, :], _f(adT_ps[0]))
                nc.vector.tensor_copy(adT[:, 2:4, :], _f(adT_ps[1]))

                for h in range(4):
                    for mc, p in [(0, 128), (1, D1)]:
                        nc.tensor.matmul(
                            pd_ps[mc][:p, h * 64:(h + 1) * 64],
                            adT[:, h, mc * 128:mc * 128 + p],
                            Vd[:, h * 64:(h + 1) * 64],
                            start=False, stop=(h == 3),
                            skip_group_check=True)

                # --- out_d = LN(proj + ctx) ---
                stats_od = ln_stats([(pd_ps[0][:, :], 128),
                                     (pd_ps[1][:D1, :], D1)], f"lnO{b % 2}")
                od = wk.tile([128, 2, 256], F32R, tag="od")
                ln_norm(od[:, 0, :], pd_ps[0][:, :], *stats_od[0])
                ln_norm(od[:D1, 1, :], pd_ps[1][:D1, :], *stats_od[1])
                if od_aff:
                    for c, p in [(0, 128), (1, D1)]:
                        nc.vector.tensor_mul(od[:p, c, :], _f(od[:p, c, :]),
                                             aff["odg"][:p, :])
                        nc.vector.tensor_add(od[:p, c, :], _f(od[:p, c, :]),
                                             aff["odb"][:p, :])
                nc.sync.dma_start(out=d_outd[b, 0:128, :], in_=od[:, 0, :])
                nc.sync.dma_start(out=d_outd[b, 128:D, :], in_=od[:D1, 1, :])

                # --- d_vec (mean over desc tokens) ---
                dv_ps = ppmm.tile([1, 256], F32, tag="mm")
                nc.tensor.matmul(dv_ps, onesc, od[:, 0, :],
                                 start=True, stop=False)
                nc.tensor.matmul(dv_ps, onesc[:D1], od[:D1, 1, :],
                                 start=False, stop=True)
                dv = wk.tile([1, 256], F32, tag="dv")
                nc.vector.tensor_scalar_mul(dv, dv_ps, 1.0 / D)
                nc.sync.dma_start(out=d_dvec[b:b + 1, :], in_=dv)

    nc.finalize()
    return nc


def _host_prep(inputs):
    """Fold params host-side; build per-core input maps."""
    g = {k: np.asarray(v, dtype=np.float32) if np.asarray(v).dtype != bool
         else np.asarray(v) for k, v in inputs.items()}

    gn, bn = g["ln_node_g"], g["ln_node_b"]
    gd, bd = g["ln_desc_g"], g["ln_desc_b"]
    gt, bt = g["tok_ln_g"], g["tok_ln_b"]

    scale = 1.0 / np.sqrt(DH)
    eff = {}
    eff["wq_g"] = (gn[:, None] * g["Wq_g"]) * scale
    eff["bq_g"] = (g["bq_g"] + bn @ g["Wq_g"]) * scale
    eff["wk_d"] = gn[:, None] * g["Wk_d"]
    eff["bk_d"] = g["bk_d"] + bn @ g["Wk_d"]
    eff["wv_d"] = gn[:, None] * g["Wv_d"]
    bv_d = g["bv_d"] + bn @ g["Wv_d"]
    eff["wk_g"] = gd[:, None] * g["Wk_g"]
    eff["bk_g"] = g["bk_g"] + bd @ g["Wk_g"]
    eff["wv_g"] = gd[:, None] * g["Wv_g"]
    bv_g = g["bv_g"] + bd @ g["Wv_g"]
    eff["wq_d"] = (gd[:, None] * g["Wq_d"]) * scale
    eff["bq_d"] = (g["bq_d"] + bd @ g["Wq_d"]) * scale
    eff["pdw"] = gt[:, None] * g["proj_desc_w"]
    pd_b = g["proj_desc_b"] + bt @ g["proj_desc_w"]
    eff["pnw"] = g["proj_node_w"]
    pn_b = g["proj_node_b"]

    cbg = (bv_g + pn_b).reshape(1, 256)
    cbd = (bv_d + pd_b).reshape(1, 256)
    id2 = g["id_emb"] + g["val_b2"][None, :]

    tok_aff = not (np.all(gt == 1.0) and np.all(bt == 0.0))
    og_aff = not (np.all(g["ln_out_g_g"] == 1.0) and np.all(g["ln_out_g_b"] == 0.0))
    od_aff = not (np.all(g["ln_out_d_g"] == 1.0) and np.all(g["ln_out_d_b"] == 0.0))

    mask = np.asarray(inputs["node_pad_mask"]).astype(bool)       # (B, N)
    madd = np.where(mask, np.float32(-np.inf), np.float32(0.0)).astype(np.float32)
    valid = (~mask).astype(np.float32)                            # (B, N)
    rden = (1.0 / np.clip(valid.sum(1), 1.0, None)).astype(np.float32)

    statics = {
        "ones": np.ones((128, 512), np.float32),
        "eye": np.eye(128, dtype=np.float32),
        "eyer": np.eye(128, dtype=np.float32),
        "cbg": np.ascontiguousarray(cbg),
        "cbd": np.ascontiguousarray(cbd),
        "vw1": np.ascontiguousarray(g["val_w1"].reshape(1, 128)),
        "vw2": np.ascontiguousarray(g["val_w2"]),
        "b1": np.ascontiguousarray(g["val_b1"].reshape(128, 1)),
        "id2a": np.ascontiguousarray(id2[0:128]),
        "id2b": np.ascontiguousarray(id2[128:D]),
    }
    for nme, key in [("wq_g", "wq_g"), ("wk_g", "wk_g"), ("wv_g", "wv_g"),
                     ("wq_d", "wq_d"), ("wk_d", "wk_d"), ("wv_d", "wv_d"),
                     ("pnw", "pnw"), ("pdw", "pdw")]:
        statics[nme] = np.ascontiguousarray(eff[key].astype(np.float32))
    for nme, key in [("bq_g", "bq_g"), ("bk_g", "bk_g"),
                     ("bq_d", "bq_d"), ("bk_d", "bk_d")]:
        statics["b" + nme[1:]] = np.ascontiguousarray(
            eff[key].astype(np.float32).reshape(256, 1))
    if tok_aff:
        statics["tokg"] = gt.reshape(1, 256).astype(np.float32)
        statics["tokb"] = bt.reshape(1, 256).astype(np.float32)
    if og_aff:
        statics["ogg"] = g["ln_out_g_g"].reshape(1, 256).astype(np.float32)
        statics["ogb"] = g["ln_out_g_b"].reshape(1, 256).astype(np.float32)
    if od_aff:
        statics["odg"] = g["ln_out_d_g"].reshape(1, 256).astype(np.float32)
        statics["odb"] = g["ln_out_d_b"].reshape(1, 256).astype(np.float32)

    nt = np.asarray(inputs["node_tokens"], dtype=np.float32)
    desc = np.asarray(inputs["desc"], dtype=np.float32)

    in_maps = []
    for c in range(NCORES):
        sl = slice(c * BB, (c + 1) * BB)
        m = dict(statics)
        m["nt"] = np.ascontiguousarray(nt[sl])
        m["desc"] = np.ascontiguousarray(desc[sl].reshape(1, BB, D))
        m["madd"] = np.ascontiguousarray(madd[sl].reshape(1, BB, N))
        m["valid"] = np.ascontiguousarray(valid[sl].T)
        m["rden"] = np.ascontiguousarray(rden[sl].reshape(1, BB))
        in_maps.append(m)

    return in_maps, (tok_aff, og_aff, od_aff)


def kernel(**inputs):
    global LAST_RESULTS
    in_maps, flags = _host_prep(inputs)

    if flags not in _PROGRAM_CACHE:
        _PROGRAM_CACHE[flags] = build_program(*flags)
    nc = _PROGRAM_CACHE[flags]

    res = run_bass_kernel_spmd(nc, in_maps, core_ids=list(range(NCORES)))
    LAST_RESULTS = res

    outs = {k: np.concatenate([res.results[c][k] for c in range(NCORES)], axis=0)
            for k in ["g_vec", "d_vec", "out_g", "out_d",
                      "scores_g", "attn_g", "scores_d", "attn_d"]}
    return (outs["g_vec"], outs["d_vec"], outs["out_g"], outs["out_d"],
            outs["scores_g"], outs["attn_g"], outs["scores_d"], outs["attn_d"])
